# revision 1
# baseline (speedup 1.0000x reference)
"""CTRNN forward kernel for Trainium2 (8 NeuronCores, batch-sharded).

Model (per step t):
    pre = x_t @ w_in^T + b_in + h @ w_hh^T + b_hh + sigma * n_t
    h'  = (1-a)*h + a*relu(pre)

For w_hh = d*I (uniform diagonal, the reset_parameters init) the recurrence is
elementwise:
    h' = (1-a)*h + relu(a*d*h + v)        with v = a*(x w^T + b + sigma n)
       = max(((1-a)+a*d)*h + v, (1-a)*h)
i.e. two fused scalar_tensor_tensor DVE ops per step.

Pipeline per 8-step block (batch-sharded 16 rows/core):
  1. one DMA loads the host-packed augmented input block x_aug^T
     (x channels + pre-scaled noise/bias channels, contraction on partitions)
  2. 8 accumulating PE matmuls against W_aug = [a*w_in^T ; I] -> PSUM v tile
     in (step,batch) x channel layout
  3. ScalarE evacuates PSUM -> SBUF
  4. the tile round-trips through a DRAM scratch: stored in matmul layout,
     re-loaded in recurrence layout (batch,chan-chunk) x (step,chan).  The
     cross-partition shuffle is unavoidable (matmul packs time x batch on
     partitions, the elementwise recurrence needs batch x channel to use all
     128 lanes); DRAM round-trip does it with 2 large DMAs per 4 blocks
     instead of 8 small SBUF-SBUF DMAs per block (HWDGE issue-bound).
  5. DVE recurrence: 2 fused ops per step
  6. one DMA stores 4 blocks of hidden states

All remaining layout work (transposes, shuffles, scale folding) happens on the
host in numpy.
"""

import os
import sys

import numpy as np

for _p in ("/opt/trn_rl_repo", os.path.expanduser("~/.axon_site/_ro/trn_rl_repo")):
    if os.path.isdir(_p) and _p not in sys.path:
        sys.path.insert(0, _p)

S, B, I, H = 1024, 128, 512, 512
TAU, DT = 100.0, 20.0
ALPHA = DT / TAU  # 0.2
SIGMA_REC = 0.05
SIGMA = float(np.sqrt(2.0 / ALPHA) * SIGMA_REC)

NCORES = 8
BL = B // NCORES  # 16 batch rows per core
SLO = 128 // BL  # 8 steps per partition-block
SHI = S // SLO  # 128 blocks
KA = I + H  # augmented contraction dim (x channels + noise channels)
KC = KA // 128  # 8 K-chunks
JC = 128 // BL  # 8 column-chunks of H on partitions
JL = H // JC  # 64 channels per chunk
SBK = 4  # blocks per superblock (DMA batching)

# dtype knobs: x/w feed the PE (bfloat16 or float32r); u is the DRAM
# round-trip scratch (bfloat16 or float32).
X_DT = os.environ.get("CTRNN_X_DT", "bfloat16")
U_DT = os.environ.get("CTRNN_U_DT", "bfloat16")
O_DT = os.environ.get("CTRNN_O_DT", "bfloat16")
FUSED = os.environ.get("CTRNN_FUSED", "1") == "1"

_PROGRAM_CACHE: dict = {}
_CTRNN_OP = None


def _get_ctrnn_dve_op():
    """Register a custom fused DVE op: out = max(in0*s0 + in1, in0*s1).

    One DVE instruction per recurrence step instead of two
    scalar_tensor_tensor ops.  Registered at runtime through the same
    table-generation path the stock ant ops use.
    """
    global _CTRNN_OP
    if _CTRNN_OP is not None:
        return _CTRNN_OP
    import concourse.dve_ops as dve_ops
    from concourse.dve_spec import C0, C1, Spec, Src0, Src1, _has_src1, lower, maxx
    from concourse.dve_uop import DveOpSpec

    name = "CTRNN_STEP_ANT"
    spec = Spec(
        body=maxx(Src0 * C0 + Src1, Src0 * C1),
        reference=lambda in0, in1, s0, s1, imm2: np.maximum(
            in0.astype(np.float32) * s0 + in1.astype(np.float32), in0 * s1
        ).astype(np.float32),
    )
    row = max(dve_ops._SUB_OPCODE_FOR_NAME.values()) + 1
    assert row < 0x20
    dve_ops._SUB_OPCODE_FOR_NAME[name] = row
    shas = {}
    for ver in ("v3", "v4"):
        try:
            shas[ver] = DveOpSpec(
                name=name, opcode=row, uops=lower(spec, ver=ver),
                rd1_en=_has_src1(spec),
            ).sha(ver)
        except Exception:
            pass
    op = dve_ops.DveOp(name, spec, subdim=False, uops_sha=shas)
    dve_ops.OPS.append(op)
    dve_ops.CUSTOM_DVE_SPECS[name] = spec
    _CTRNN_OP = op
    return op


def _build_program(n_shi: int, coef_a: float, coef_c: float, x_dt_name: str,
                   u_dt_name: str, o_dt_name: str = "float32"):
    import concourse.bacc as bacc
    import concourse.mybir as mybir
    from concourse import tile

    f32 = mybir.dt.float32
    x_dt = getattr(mybir.dt, x_dt_name)
    u_dt = getattr(mybir.dt, u_dt_name)
    o_dt = getattr(mybir.dt, o_dt_name)
    n_sb = n_shi // SBK
    assert n_sb * SBK == n_shi

    nc = bacc.Bacc(
        "TRN2",
        target_bir_lowering=False,
        debug=False,
        num_devices=NCORES,
    )

    # x_aug layout (kc, p, s_hi, f): per (kc,p) the (s_hi, f) plane is
    # contiguous, so superblock loads read SBK*128-element runs (>=512B even
    # in bf16 -> no DMA read-modify-write penalty).
    x_d = nc.dram_tensor(
        "x_aug", [KC, 128, n_shi, SLO * BL], x_dt, kind="ExternalInput"
    )
    w_d = nc.dram_tensor("w_aug", [KA, H], x_dt, kind="ExternalInput")
    h0_d = nc.dram_tensor("h0l", [128, JL], f32, kind="ExternalInput")
    o_d = nc.dram_tensor("out_l", [n_shi, 128, SLO * JL], o_dt, kind="ExternalOutput")
    u_d = nc.dram_tensor("u_scr", [n_shi, 128, H], u_dt, kind="Internal")

    mult = mybir.AluOpType.mult
    add = mybir.AluOpType.add
    amax = mybir.AluOpType.max
    copy_fn = mybir.ActivationFunctionType.Copy

    with tile.TileContext(nc) as tc:
        with (
            tc.tile_pool(name="const", bufs=1) as cpool,
            tc.tile_pool(name="xp", bufs=4) as xpool,
            tc.tile_pool(name="ps", bufs=8, space="PSUM") as ppool,
            tc.tile_pool(name="up", bufs=10) as upool,
            tc.tile_pool(name="vp", bufs=4) as vpool,
            tc.tile_pool(name="op", bufs=4) as opool,
            tc.tile_pool(name="sp", bufs=2) as spool,
        ):
            # Weights resident in SBUF: [i_within_chunk, (kc, j)]
            w_sb = cpool.tile([128, KC * H], x_dt)
            nc.sync.dma_start(
                out=w_sb[:].rearrange("p (kc n) -> p kc n", kc=KC),
                in_=w_d.ap().rearrange("(kc p) n -> p kc n", p=128),
            )
            h0_sb = cpool.tile([128, JL], f32)
            nc.sync.dma_start(out=h0_sb[:], in_=h0_d.ap())

            prev = h0_sb[:]
            for sb in range(n_sb):
                base = sb * SBK
                # ---- one DMA: 4 blocks of x_aug^T; SBUF free order (kc, blk, f)
                x_sb = xpool.tile([128, SBK * KC * SLO * BL], x_dt)
                nc.sync.dma_start(
                    out=x_sb[:],
                    in_=x_d.ap()[:, :, base : base + SBK, :].rearrange(
                        "kc p blk f -> p kc blk f"
                    ),
                )

                # ---- per block: matmul + evacuate + store to scratch
                for blk in range(SBK):
                    shi = base + blk
                    ps = ppool.tile([128, H], f32)
                    for kc in range(KC):
                        nc.tensor.matmul(
                            out=ps[:],
                            lhsT=x_sb[
                                :, (kc * SBK + blk) * 128 : (kc * SBK + blk + 1) * 128
                            ],
                            rhs=w_sb[:, kc * H : (kc + 1) * H],
                            start=(kc == 0),
                            stop=(kc == KC - 1),
                        )
                    u_sb = upool.tile([128, H], u_dt)
                    nc.scalar.activation(out=u_sb[:], in_=ps[:], func=copy_fn)
                    # issue the scratch store from the ACT HWDGE ring to keep
                    # the SP sequencer free for the big loads/stores
                    nc.scalar.dma_start(out=u_d.ap()[shi], in_=u_sb[:])

                # ---- one DMA: reload 4 blocks in recurrence layout
                # v[(b,jc), (blk,sl,j)] = u_scr[base+blk, (sl,b), (jc,j)]
                v_sb = vpool.tile([128, SBK * SLO * JL], u_dt)
                nc.sync.dma_start(
                    out=v_sb[:],
                    in_=u_d.ap()[base : base + SBK].rearrange(
                        "blk (sl b) (jc j) -> b jc blk sl j", b=BL, jc=JC
                    ),
                )

                # ---- recurrence: h' = max(a*h + v, c*h)
                # (state tile stays fp32 -- it is re-read every step)
                o_sb = opool.tile([128, SBK * SLO * JL], f32)
                for st in range(SBK * SLO):
                    vsl = v_sb[:, st * JL : (st + 1) * JL]
                    osl = o_sb[:, st * JL : (st + 1) * JL]
                    if FUSED:
                        nc.vector._custom_dve(
                            _get_ctrnn_dve_op(), out=osl, in0=prev, in1=vsl,
                            s0=coef_a, s1=coef_c,
                        )
                    else:
                        p_tmp = spool.tile([128, JL], f32)
                        # p = coef_a * h + v
                        nc.vector.scalar_tensor_tensor(
                            out=p_tmp[:], in0=prev, scalar=coef_a, in1=vsl,
                            op0=mult, op1=add,
                        )
                        # h' = max(coef_c * h, p)
                        nc.vector.scalar_tensor_tensor(
                            out=osl, in0=prev, scalar=coef_c, in1=p_tmp[:],
                            op0=mult, op1=amax,
                        )
                    prev = osl

                # ---- one DMA: store 4 blocks of hidden states
                # (SWDGE when the output dtype needs a cast)
                store_eng = nc.sync if o_dt == f32 else nc.gpsimd
                store_eng.dma_start(
                    out=o_d.ap()[base : base + SBK].rearrange(
                        "blk p f -> p blk f"
                    ),
                    in_=o_sb[:],
                )

    nc.finalize()
    return nc


def _get_program(n_shi, coef_a, coef_c, x_dt_name, u_dt_name, o_dt_name="float32"):
    key = (n_shi, coef_a, coef_c, x_dt_name, u_dt_name, o_dt_name)
    if key not in _PROGRAM_CACHE:
        _PROGRAM_CACHE[key] = _build_program(
            n_shi, coef_a, coef_c, x_dt_name, u_dt_name, o_dt_name
        )
    return _PROGRAM_CACHE[key]


def _np_dt(name):
    if name in ("float32", "float32r"):
        return np.float32
    import ml_dtypes

    return np.dtype(ml_dtypes.bfloat16)


def _host_inputs(x, noise, w_in, b_in, b_hh, h0, x_np_dt):
    """Per-core input dicts (all the layout work, done on the host)."""
    bias = (ALPHA * (b_in + b_hh)).astype(np.float32)
    w_aug = np.concatenate(
        [ALPHA * w_in.T.astype(np.float32), np.eye(H, dtype=np.float32)], axis=0
    ).astype(x_np_dt)

    in_maps = []
    for c in range(NCORES):
        bs = slice(c * BL, (c + 1) * BL)
        x_c = x[:, bs, :].reshape(SHI, SLO, BL, I).transpose(0, 3, 1, 2)
        n_c = (ALPHA * SIGMA) * noise[:, bs, :] + bias
        n_c = n_c.astype(np.float32).reshape(SHI, SLO, BL, H).transpose(0, 3, 1, 2)
        x_aug = np.concatenate(
            [x_c.astype(x_np_dt), n_c.astype(x_np_dt)], axis=1
        )  # (SHI, KA, SLO, BL)
        x_aug = (
            x_aug.reshape(SHI, KC, 128, SLO * BL)
            .transpose(1, 2, 0, 3)  # (kc, p, s_hi, f)
        )
        x_aug = np.ascontiguousarray(x_aug)
        h0_l = h0[bs].astype(np.float32).reshape(128, JL)  # p = b*JC + jc
        in_maps.append(
            {"x_aug": x_aug, "w_aug": w_aug, "h0l": np.ascontiguousarray(h0_l)}
        )
    return in_maps


def _gather_output(results):
    out = np.empty((S, B, H), dtype=np.float32)
    for c in range(NCORES):
        o = np.asarray(results[c]["out_l"], dtype=np.float32)
        o = (
            o.reshape(SHI, BL, JC, SLO, JL)
            .transpose(0, 3, 1, 2, 4)
            .reshape(S, BL, H)
        )
        out[:, c * BL : (c + 1) * BL, :] = o
    return out


def _numpy_fallback(x, noise, w_in, b_in, w_hh, b_hh, h0):
    h = h0.astype(np.float32).copy()
    out = np.empty((S, B, H), dtype=np.float32)
    one_minus_a = np.float32(1.0 - ALPHA)
    a = np.float32(ALPHA)
    sg = np.float32(SIGMA)
    for t in range(S):
        pre = x[t] @ w_in.T + b_in + h @ w_hh.T + b_hh + sg * noise[t]
        h = h * one_minus_a + np.maximum(pre, 0) * a
        out[t] = h
    return out


def kernel(x, noise, w_in, b_in, w_hh, b_hh, h0):
    x = np.asarray(x, dtype=np.float32)
    noise = np.asarray(noise, dtype=np.float32)
    w_in = np.asarray(w_in, dtype=np.float32)
    b_in = np.asarray(b_in, dtype=np.float32)
    w_hh = np.asarray(w_hh, dtype=np.float32)
    b_hh = np.asarray(b_hh, dtype=np.float32)
    h0 = np.asarray(h0, dtype=np.float32)

    d = np.diagonal(w_hh)
    uniform_diag = np.all(w_hh == np.diag(d)) and np.all(d == d[0])
    if not uniform_diag:
        return _numpy_fallback(x, noise, w_in, b_in, w_hh, b_hh, h0)

    dval = float(d[0])
    coef_a = (1.0 - ALPHA) + ALPHA * dval  # 0.9 for d=0.5
    coef_c = 1.0 - ALPHA  # 0.8

    from concourse.bass_utils import run_bass_kernel_spmd

    nc = _get_program(SHI, coef_a, coef_c, X_DT, U_DT, O_DT)
    in_maps = _host_inputs(x, noise, w_in, b_in, b_hh, h0, _np_dt(X_DT))
    res = run_bass_kernel_spmd(nc, in_maps, list(range(NCORES)))
    return _gather_output(res.results)



# revision 7
# speedup vs baseline: 1.6421x; 1.6421x over previous
"""CTRNN forward kernel for Trainium2 (8 NeuronCores, batch-sharded).

Model (per step t):
    pre = x_t @ w_in^T + b_in + h @ w_hh^T + b_hh + sigma * n_t
    h'  = (1-a)*h + a*relu(pre)

For w_hh = d*I (the reset_parameters init) the recurrence is elementwise:
    h' = max(((1-a)+a*d)*h + v, (1-a)*h)   with v = a*(x w^T + b + sigma n)
i.e. one fused DVE op per step.

Layout trick: the matmul is emitted TRANSPOSED (out = v^T) so PSUM comes out
directly in recurrence layout -- partition = h_lo (channel % 128), free =
(h_hi, sl, b).  The elementwise recurrence then runs on all 128 lanes with no
cross-partition shuffle, eliminating the DRAM scratch round-trip a
(step,batch)-partition matmul layout would force.

Noise + bias enter through a zero-compute path: the PSUM tile is evacuated by
ScalarE to SBUF (bf16), then one SWDGE accumulate-DMA adds the host-prescaled
noise block in-place (accum_op=add).  No PE identity-matmul (which would
double PE work), no Pool/DVE elementwise adds.

Per 4-block superblock (32 steps), software-pipelined one superblock deep so
the in-order Pool DMA queue never back-pressures the DVE chain:
  SP ring:    1 DMA  x superblock load (bf16)
  PE:         16 matmuls/block (contraction I=512 over 4 k-chunks x 4 h-chunks)
  ACT:        1 evac/block  PSUM f32 -> SBUF bf16
  Pool ring:  1 accum-DMA (u += noise), 1 output store (f32 -> bf16 cast)
  DVE:        32 fused recurrence steps (the critical path, ~127 ns each)
"""

import os
import sys

import numpy as np

for _p in ("/opt/trn_rl_repo", os.path.expanduser("~/.axon_site/_ro/trn_rl_repo")):
    if os.path.isdir(_p) and _p not in sys.path:
        sys.path.insert(0, _p)

S, B, I, H = 1024, 128, 512, 512
TAU, DT = 100.0, 20.0
ALPHA = DT / TAU  # 0.2
SIGMA_REC = 0.05
SIGMA = float(np.sqrt(2.0 / ALPHA) * SIGMA_REC)

NCORES = 8
BL = B // NCORES  # 16 batch rows per core
SLO = 128 // BL  # 8 steps per partition-block
SHI = S // SLO  # 128 blocks
KCX = I // 128  # 4 contraction chunks
HH = H // 128  # 4 channel chunks (h_hi)
SBK = 4  # blocks per superblock
NSB = SHI // SBK  # 32 superblocks
FB = SLO * BL  # 128 = free size of one block (sl, b)
UF = SBK * H  # 2048 = u/o tile free size per superblock

_PROGRAM_CACHE: dict = {}
_CTRNN_OP = None


def _get_ctrnn_dve_op():
    """Register a custom fused DVE op: out = max(in0*s0 + in1, in0*s1)."""
    global _CTRNN_OP
    if _CTRNN_OP is not None:
        return _CTRNN_OP
    import concourse.dve_ops as dve_ops
    from concourse.dve_spec import C0, C1, Spec, Src0, Src1, _has_src1, lower, maxx
    from concourse.dve_uop import DveOpSpec

    name = "CTRNN_STEP_ANT"
    spec = Spec(
        body=maxx(Src0 * C0 + Src1, Src0 * C1),
        reference=lambda in0, in1, s0, s1, imm2: np.maximum(
            in0.astype(np.float32) * s0
            + np.asarray(in1).reshape(np.asarray(in0).shape).astype(np.float32),
            in0 * s1,
        ).astype(np.float32),
    )
    row = max(dve_ops._SUB_OPCODE_FOR_NAME.values()) + 1
    assert row < 0x20
    dve_ops._SUB_OPCODE_FOR_NAME[name] = row
    shas = {}
    for ver in ("v3", "v4"):
        try:
            shas[ver] = DveOpSpec(
                name=name, opcode=row, uops=lower(spec, ver=ver),
                rd1_en=_has_src1(spec),
            ).sha(ver)
        except Exception:
            pass
    op = dve_ops.DveOp(name, spec, subdim=False, uops_sha=shas)
    dve_ops.OPS.append(op)
    dve_ops.CUSTOM_DVE_SPECS[name] = spec
    _CTRNN_OP = op
    return op


def _build_program(n_sb: int, coef_a: float, coef_c: float):
    import concourse.bacc as bacc
    import concourse.mybir as mybir
    from concourse import tile

    f32 = mybir.dt.float32
    bf16 = mybir.dt.bfloat16
    copy_fn = mybir.ActivationFunctionType.Copy
    add = mybir.AluOpType.add
    op = _get_ctrnn_dve_op()

    nc = bacc.Bacc(
        "TRN2",
        target_bir_lowering=False,
        debug=False,
        num_devices=NCORES,
    )

    # x_l[kc, k_lo, sb, (blk, sl, b)] -- per (kc,k_lo) the innermost run is
    # SBK*FB elems (1 KiB bf16 per (kc,p,sb)), contiguous over (blk, sl, b).
    x_d = nc.dram_tensor("x_l", [KCX, 128, n_sb, SBK * FB], bf16, kind="ExternalInput")
    # w_l[kc, k_lo, h] = ALPHA * w_in[h, kc*128+k_lo]
    w_d = nc.dram_tensor("w_l", [KCX, 128, H], bf16, kind="ExternalInput")
    # n_l[sb, h_lo, (blk, h_hi, sl, b)] = ALPHA*(SIGMA*noise + bias), bf16
    n_d = nc.dram_tensor("n_l", [n_sb, 128, UF], bf16, kind="ExternalInput")
    # h0_l[h_lo, (h_hi, b)]
    h0_d = nc.dram_tensor("h0_l", [128, HH * BL], f32, kind="ExternalInput")
    # out_l[sb, h_lo, (blk, sl, h_hi, b)]
    o_d = nc.dram_tensor("out_l", [n_sb, 128, UF], bf16, kind="ExternalOutput")

    with tile.TileContext(nc) as tc:
        with (
            tc.tile_pool(name="const", bufs=1) as cpool,
            tc.tile_pool(name="xp", bufs=3) as xpool,
            tc.tile_pool(name="ps", bufs=8, space="PSUM") as ppool,
            tc.tile_pool(name="up", bufs=3) as upool,
            tc.tile_pool(name="op", bufs=3) as opool,
        ):
            w_sb = cpool.tile([128, KCX * H], bf16)
            nc.sync.dma_start(
                out=w_sb[:].rearrange("p (kc h) -> p kc h", kc=KCX),
                in_=w_d.ap().rearrange("kc p h -> p kc h"),
            )
            h0_sb = cpool.tile([128, HH * BL], f32)
            nc.sync.dma_start(out=h0_sb[:], in_=h0_d.ap())

            def load_x(sb):
                x_sb = xpool.tile([128, KCX * SBK * FB], bf16)
                nc.sync.dma_start(
                    out=x_sb[:].rearrange("p (kc f) -> p kc f", kc=KCX),
                    in_=x_d.ap()[:, :, sb, :].rearrange("kc p f -> p kc f"),
                )
                return x_sb

            def mm_evac_accum(sb, x_sb):
                """PE matmuls + ACT evac for all SBK blocks, then the noise
                accumulate-DMA. Returns the u tile."""
                u_sb = upool.tile([128, UF], bf16)
                for blk in range(SBK):
                    ps = ppool.tile([128, H], f32)
                    for hh in range(HH):
                        for kc in range(KCX):
                            nc.tensor.matmul(
                                out=ps[:, hh * 128 : (hh + 1) * 128],
                                lhsT=w_sb[:, kc * H + hh * 128 : kc * H + hh * 128 + 128],
                                rhs=x_sb[:, (kc * SBK + blk) * FB : (kc * SBK + blk + 1) * FB],
                                start=(kc == 0),
                                stop=(kc == KCX - 1),
                            )
                    nc.scalar.activation(
                        out=u_sb[:, blk * H : (blk + 1) * H], in_=ps[:], func=copy_fn
                    )
                # u += noise (SWDGE accumulate; adds bias+noise in one DMA)
                nc.gpsimd.dma_start(out=u_sb[:], in_=n_d.ap()[sb], accum_op=add)
                return u_sb

            prev = h0_sb[:]
            x_next = load_x(0)
            u_next = mm_evac_accum(0, x_next)
            for sb in range(n_sb):
                u_sb = u_next
                # free dims of u: (blk, h_hi, sl, b) -> index (blk, sl)
                u_v = u_sb[:].rearrange(
                    "p (blk hh sl b) -> p blk sl hh b", blk=SBK, hh=HH, b=BL
                )
                if sb + 1 < n_sb:
                    x_next = load_x(sb + 1)
                    u_next = mm_evac_accum(sb + 1, x_next)
                o_sb = opool.tile([128, UF], f32)
                for blk in range(SBK):
                    for sl in range(SLO):
                        st = blk * SLO + sl
                        osl = o_sb[:, st * HH * BL : (st + 1) * HH * BL]
                        nc.vector._custom_dve(
                            op, out=osl, in0=prev, in1=u_v[:, blk, sl],
                            s0=coef_a, s1=coef_c,
                        )
                        prev = osl
                # store (f32 -> bf16 cast on the SWDGE path)
                nc.gpsimd.dma_start(out=o_d.ap()[sb], in_=o_sb[:])

    nc.finalize()
    return nc


def _get_program(n_sb, coef_a, coef_c):
    key = (n_sb, coef_a, coef_c)
    if key not in _PROGRAM_CACHE:
        _PROGRAM_CACHE[key] = _build_program(n_sb, coef_a, coef_c)
    return _PROGRAM_CACHE[key]


def _bf16():
    import ml_dtypes

    return np.dtype(ml_dtypes.bfloat16)


def _pack_core(x_c, n_hat_c, h0_c, n_sb):
    """Device-layout input arrays for one core.

    x_c: (S_c, BL, I) f32;  n_hat_c: (S_c, BL, H) f32 (prescaled noise+bias);
    h0_c: (BL, H) f32.
    """
    bf = _bf16()
    s_c = n_sb * SBK * SLO
    # x_l[kc, k_lo, sb, (blk, sl, b)]
    x_l = (
        x_c.reshape(n_sb, SBK, SLO, BL, KCX, 128)
        .transpose(4, 5, 0, 1, 2, 3)
        .reshape(KCX, 128, n_sb, SBK * FB)
    )
    # n_l[sb, h_lo, (blk, h_hi, sl, b)]
    n_l = (
        n_hat_c.reshape(n_sb, SBK, SLO, BL, HH, 128)
        .transpose(0, 5, 1, 4, 2, 3)
        .reshape(n_sb, 128, UF)
    )
    # h0_l[h_lo, (h_hi, b)]
    h0_l = h0_c.reshape(BL, HH, 128).transpose(2, 1, 0).reshape(128, HH * BL)
    assert s_c == x_c.shape[0]
    return {
        "x_l": np.ascontiguousarray(x_l.astype(bf)),
        "n_l": np.ascontiguousarray(n_l.astype(bf)),
        "h0_l": np.ascontiguousarray(h0_l.astype(np.float32)),
    }


def _unpack_out(o, n_sb):
    """out_l[sb, h_lo, (blk, sl, h_hi, b)] -> (S_c, BL, H) f32."""
    o = np.asarray(o, dtype=np.float32)
    return (
        o.reshape(n_sb, 128, SBK, SLO, HH, BL)
        .transpose(0, 2, 3, 5, 4, 1)
        .reshape(n_sb * SBK * SLO, BL, H)
    )


def _numpy_fallback(x, noise, w_in, b_in, w_hh, b_hh, h0):
    h = h0.astype(np.float32).copy()
    out = np.empty((S, B, H), dtype=np.float32)
    one_minus_a = np.float32(1.0 - ALPHA)
    a = np.float32(ALPHA)
    sg = np.float32(SIGMA)
    for t in range(S):
        pre = x[t] @ w_in.T + b_in + h @ w_hh.T + b_hh + sg * noise[t]
        h = h * one_minus_a + np.maximum(pre, 0) * a
        out[t] = h
    return out


def kernel(x, noise, w_in, b_in, w_hh, b_hh, h0):
    x = np.asarray(x, dtype=np.float32)
    noise = np.asarray(noise, dtype=np.float32)
    w_in = np.asarray(w_in, dtype=np.float32)
    b_in = np.asarray(b_in, dtype=np.float32)
    w_hh = np.asarray(w_hh, dtype=np.float32)
    b_hh = np.asarray(b_hh, dtype=np.float32)
    h0 = np.asarray(h0, dtype=np.float32)

    d = np.diagonal(w_hh)
    uniform_diag = np.all(w_hh == np.diag(d)) and np.all(d == d[0])
    if not uniform_diag:
        return _numpy_fallback(x, noise, w_in, b_in, w_hh, b_hh, h0)

    dval = float(d[0])
    coef_a = (1.0 - ALPHA) + ALPHA * dval  # 0.9 for d=0.5
    coef_c = 1.0 - ALPHA  # 0.8

    from concourse.bass_utils import run_bass_kernel_spmd

    nc = _get_program(NSB, coef_a, coef_c)

    bf = _bf16()
    bias = (ALPHA * (b_in + b_hh)).astype(np.float32)
    w_l = np.ascontiguousarray(
        (ALPHA * w_in.T).reshape(KCX, 128, H).astype(bf)
    )
    in_maps = []
    for c in range(NCORES):
        bs = slice(c * BL, (c + 1) * BL)
        n_hat = (ALPHA * SIGMA) * noise[:, bs, :] + bias
        m = _pack_core(x[:, bs, :], n_hat.astype(np.float32), h0[bs], NSB)
        m["w_l"] = w_l
        in_maps.append(m)

    res = run_bass_kernel_spmd(nc, in_maps, list(range(NCORES)))

    out = np.empty((S, B, H), dtype=np.float32)
    for c in range(NCORES):
        out[:, c * BL : (c + 1) * BL, :] = _unpack_out(res.results[c]["out_l"], NSB)
    return out


# revision 9
# speedup vs baseline: 1.6665x; 1.0149x over previous
"""CTRNN forward kernel for Trainium2 (8 NeuronCores, batch-sharded).

Model (per step t):
    pre = x_t @ w_in^T + b_in + h @ w_hh^T + b_hh + sigma * n_t
    h'  = (1-a)*h + a*relu(pre)

For w_hh = d*I (the reset_parameters init) the recurrence is elementwise:
    h' = max(((1-a)+a*d)*h + v, (1-a)*h)   with v = a*(x w^T + b + sigma n)
i.e. one fused DVE op per step.

Layout trick: the matmul is emitted TRANSPOSED (out = v^T) so PSUM comes out
directly in recurrence layout -- partition = h_lo (channel % 128), free =
(h_hi, sl, b).  The elementwise recurrence then runs on all 128 lanes with no
cross-partition shuffle, eliminating the DRAM scratch round-trip a
(step,batch)-partition matmul layout would force.

Noise + bias enter through a zero-compute path: the PSUM tile is evacuated by
ScalarE to SBUF (bf16), then one SWDGE accumulate-DMA adds the host-prescaled
noise block in-place (accum_op=add).  No PE identity-matmul (which would
double PE work), no Pool/DVE elementwise adds.

Per 4-block superblock (32 steps), software-pipelined one superblock deep so
the in-order Pool DMA queue never back-pressures the DVE chain:
  SP ring:    1 DMA  x superblock load (bf16)
  PE:         16 matmuls/block (contraction I=512 over 4 k-chunks x 4 h-chunks)
  ACT:        1 evac/block  PSUM f32 -> SBUF bf16
  Pool ring:  1 accum-DMA (u += noise), 1 output store (f32 -> bf16 cast)
  DVE:        32 fused recurrence steps (the critical path, ~127 ns each)
"""

import os
import sys

import numpy as np

for _p in ("/opt/trn_rl_repo", os.path.expanduser("~/.axon_site/_ro/trn_rl_repo")):
    if os.path.isdir(_p) and _p not in sys.path:
        sys.path.insert(0, _p)

S, B, I, H = 1024, 128, 512, 512
TAU, DT = 100.0, 20.0
ALPHA = DT / TAU  # 0.2
SIGMA_REC = 0.05
SIGMA = float(np.sqrt(2.0 / ALPHA) * SIGMA_REC)

NCORES = 8
BL = B // NCORES  # 16 batch rows per core
SLO = 128 // BL  # 8 steps per partition-block
SHI = S // SLO  # 128 blocks
KCX = I // 128  # 4 contraction chunks
HH = H // 128  # 4 channel chunks (h_hi)
SBK = 4  # blocks per superblock
NSB = SHI // SBK  # 32 superblocks
FB = SLO * BL  # 128 = free size of one block (sl, b)
UF = SBK * H  # 2048 = u/o tile free size per superblock

_PROGRAM_CACHE: dict = {}
_CTRNN_OP = None


def _get_ctrnn_dve_op():
    """Register a custom fused DVE op: out = max(in0*s0 + in1, in0*s1)."""
    global _CTRNN_OP
    if _CTRNN_OP is not None:
        return _CTRNN_OP
    import concourse.dve_ops as dve_ops
    from concourse.dve_spec import C0, C1, Spec, Src0, Src1, _has_src1, lower, maxx
    from concourse.dve_uop import DveOpSpec

    name = "CTRNN_STEP_ANT"
    spec = Spec(
        body=maxx(Src0 * C0 + Src1, Src0 * C1),
        reference=lambda in0, in1, s0, s1, imm2: np.maximum(
            in0.astype(np.float32) * s0
            + np.asarray(in1).reshape(np.asarray(in0).shape).astype(np.float32),
            in0 * s1,
        ).astype(np.float32),
    )
    row = max(dve_ops._SUB_OPCODE_FOR_NAME.values()) + 1
    assert row < 0x20
    dve_ops._SUB_OPCODE_FOR_NAME[name] = row
    shas = {}
    for ver in ("v3", "v4"):
        try:
            shas[ver] = DveOpSpec(
                name=name, opcode=row, uops=lower(spec, ver=ver),
                rd1_en=_has_src1(spec),
            ).sha(ver)
        except Exception:
            pass
    op = dve_ops.DveOp(name, spec, subdim=False, uops_sha=shas)
    dve_ops.OPS.append(op)
    dve_ops.CUSTOM_DVE_SPECS[name] = spec
    _CTRNN_OP = op
    return op


def _build_program(n_sb: int, coef_a: float, coef_c: float):
    import concourse.bacc as bacc
    import concourse.mybir as mybir
    from concourse import tile

    f32 = mybir.dt.float32
    bf16 = mybir.dt.bfloat16
    copy_fn = mybir.ActivationFunctionType.Copy
    add = mybir.AluOpType.add
    op = _get_ctrnn_dve_op()

    nc = bacc.Bacc(
        "TRN2",
        target_bir_lowering=False,
        debug=False,
        num_devices=NCORES,
    )

    # x_l[kc, k_lo, sb, (blk, sl, b)] -- per (kc,k_lo) the innermost run is
    # SBK*FB elems (1 KiB bf16 per (kc,p,sb)), contiguous over (blk, sl, b).
    x_d = nc.dram_tensor("x_l", [KCX, 128, n_sb, SBK * FB], bf16, kind="ExternalInput")
    # w_l[kc, k_lo, h] = ALPHA * w_in[h, kc*128+k_lo]
    w_d = nc.dram_tensor("w_l", [KCX, 128, H], bf16, kind="ExternalInput")
    # n_l[sb, h_lo, (blk, h_hi, sl, b)] = ALPHA*(SIGMA*noise + bias), bf16
    n_d = nc.dram_tensor("n_l", [n_sb, 128, UF], bf16, kind="ExternalInput")
    # h0_l[h_lo, (h_hi, b)]
    h0_d = nc.dram_tensor("h0_l", [128, HH * BL], f32, kind="ExternalInput")
    # out_l[sb, h_lo, (blk, sl, h_hi, b)]
    o_d = nc.dram_tensor("out_l", [n_sb, 128, UF], bf16, kind="ExternalOutput")

    with tile.TileContext(nc) as tc:
        with (
            tc.tile_pool(name="const", bufs=1) as cpool,
            tc.tile_pool(name="xp", bufs=4) as xpool,
            tc.tile_pool(name="ps", bufs=8, space="PSUM") as ppool,
            tc.tile_pool(name="up", bufs=4) as upool,
            tc.tile_pool(name="op", bufs=3) as opool,
        ):
            w_sb = cpool.tile([128, KCX * H], bf16)
            nc.sync.dma_start(
                out=w_sb[:].rearrange("p (kc h) -> p kc h", kc=KCX),
                in_=w_d.ap().rearrange("kc p h -> p kc h"),
            )
            h0_sb = cpool.tile([128, HH * BL], f32)
            nc.sync.dma_start(out=h0_sb[:], in_=h0_d.ap())

            def load_x(sb):
                x_sb = xpool.tile([128, KCX * SBK * FB], bf16)
                nc.sync.dma_start(
                    out=x_sb[:].rearrange("p (kc f) -> p kc f", kc=KCX),
                    in_=x_d.ap()[:, :, sb, :].rearrange("kc p f -> p kc f"),
                )
                return x_sb

            def mm_evac_accum(sb, x_sb):
                """PE matmuls + ACT evac for all SBK blocks, plus per-block
                noise accumulate-DMAs. Returns the u tile."""
                u_sb = upool.tile([128, UF], bf16)
                for blk in range(SBK):
                    ps = ppool.tile([128, H], f32)
                    for hh in range(HH):
                        for kc in range(KCX):
                            nc.tensor.matmul(
                                out=ps[:, hh * 128 : (hh + 1) * 128],
                                lhsT=w_sb[:, kc * H + hh * 128 : kc * H + hh * 128 + 128],
                                rhs=x_sb[:, (kc * SBK + blk) * FB : (kc * SBK + blk + 1) * FB],
                                start=(kc == 0),
                                stop=(kc == KCX - 1),
                            )
                    nc.scalar.activation(
                        out=u_sb[:, blk * H : (blk + 1) * H], in_=ps[:], func=copy_fn
                    )
                    # u += noise (SWDGE accumulate; adds bias+noise in one DMA)
                    nc.gpsimd.dma_start(
                        out=u_sb[:, blk * H : (blk + 1) * H],
                        in_=n_d.ap()[sb, :, blk * H : (blk + 1) * H],
                        accum_op=add,
                    )
                return u_sb

            LOOKAHEAD = 2
            prev = h0_sb[:]
            pending = []
            for sb in range(min(LOOKAHEAD, n_sb)):
                pending.append(mm_evac_accum(sb, load_x(sb)))
            for sb in range(n_sb):
                u_sb = pending.pop(0)
                # free dims of u: (blk, h_hi, sl, b) -> index (blk, sl)
                u_v = u_sb[:].rearrange(
                    "p (blk hh sl b) -> p blk sl hh b", blk=SBK, hh=HH, b=BL
                )
                if sb + LOOKAHEAD < n_sb:
                    pending.append(mm_evac_accum(sb + LOOKAHEAD, load_x(sb + LOOKAHEAD)))
                o_sb = opool.tile([128, UF], f32)
                for blk in range(SBK):
                    for sl in range(SLO):
                        st = blk * SLO + sl
                        osl = o_sb[:, st * HH * BL : (st + 1) * HH * BL]
                        nc.vector._custom_dve(
                            op, out=osl, in0=prev, in1=u_v[:, blk, sl],
                            s0=coef_a, s1=coef_c,
                        )
                        prev = osl
                # store (f32 -> bf16 cast on the SWDGE path)
                nc.gpsimd.dma_start(out=o_d.ap()[sb], in_=o_sb[:])

    nc.finalize()
    return nc


def _get_program(n_sb, coef_a, coef_c):
    key = (n_sb, coef_a, coef_c)
    if key not in _PROGRAM_CACHE:
        _PROGRAM_CACHE[key] = _build_program(n_sb, coef_a, coef_c)
    return _PROGRAM_CACHE[key]


def _bf16():
    import ml_dtypes

    return np.dtype(ml_dtypes.bfloat16)


def _pack_core(x_c, n_hat_c, h0_c, n_sb):
    """Device-layout input arrays for one core.

    x_c: (S_c, BL, I) f32;  n_hat_c: (S_c, BL, H) f32 (prescaled noise+bias);
    h0_c: (BL, H) f32.
    """
    bf = _bf16()
    s_c = n_sb * SBK * SLO
    # x_l[kc, k_lo, sb, (blk, sl, b)]
    x_l = (
        x_c.reshape(n_sb, SBK, SLO, BL, KCX, 128)
        .transpose(4, 5, 0, 1, 2, 3)
        .reshape(KCX, 128, n_sb, SBK * FB)
    )
    # n_l[sb, h_lo, (blk, h_hi, sl, b)]
    n_l = (
        n_hat_c.reshape(n_sb, SBK, SLO, BL, HH, 128)
        .transpose(0, 5, 1, 4, 2, 3)
        .reshape(n_sb, 128, UF)
    )
    # h0_l[h_lo, (h_hi, b)]
    h0_l = h0_c.reshape(BL, HH, 128).transpose(2, 1, 0).reshape(128, HH * BL)
    assert s_c == x_c.shape[0]
    return {
        "x_l": np.ascontiguousarray(x_l.astype(bf)),
        "n_l": np.ascontiguousarray(n_l.astype(bf)),
        "h0_l": np.ascontiguousarray(h0_l.astype(np.float32)),
    }


def _unpack_out(o, n_sb):
    """out_l[sb, h_lo, (blk, sl, h_hi, b)] -> (S_c, BL, H) f32."""
    o = np.asarray(o, dtype=np.float32)
    return (
        o.reshape(n_sb, 128, SBK, SLO, HH, BL)
        .transpose(0, 2, 3, 5, 4, 1)
        .reshape(n_sb * SBK * SLO, BL, H)
    )


def _numpy_fallback(x, noise, w_in, b_in, w_hh, b_hh, h0):
    h = h0.astype(np.float32).copy()
    out = np.empty((S, B, H), dtype=np.float32)
    one_minus_a = np.float32(1.0 - ALPHA)
    a = np.float32(ALPHA)
    sg = np.float32(SIGMA)
    for t in range(S):
        pre = x[t] @ w_in.T + b_in + h @ w_hh.T + b_hh + sg * noise[t]
        h = h * one_minus_a + np.maximum(pre, 0) * a
        out[t] = h
    return out


def kernel(x, noise, w_in, b_in, w_hh, b_hh, h0):
    x = np.asarray(x, dtype=np.float32)
    noise = np.asarray(noise, dtype=np.float32)
    w_in = np.asarray(w_in, dtype=np.float32)
    b_in = np.asarray(b_in, dtype=np.float32)
    w_hh = np.asarray(w_hh, dtype=np.float32)
    b_hh = np.asarray(b_hh, dtype=np.float32)
    h0 = np.asarray(h0, dtype=np.float32)

    d = np.diagonal(w_hh)
    uniform_diag = np.all(w_hh == np.diag(d)) and np.all(d == d[0])
    if not uniform_diag:
        return _numpy_fallback(x, noise, w_in, b_in, w_hh, b_hh, h0)

    dval = float(d[0])
    coef_a = (1.0 - ALPHA) + ALPHA * dval  # 0.9 for d=0.5
    coef_c = 1.0 - ALPHA  # 0.8

    from concourse.bass_utils import run_bass_kernel_spmd

    nc = _get_program(NSB, coef_a, coef_c)

    bf = _bf16()
    bias = (ALPHA * (b_in + b_hh)).astype(np.float32)
    w_l = np.ascontiguousarray(
        (ALPHA * w_in.T).reshape(KCX, 128, H).astype(bf)
    )
    in_maps = []
    for c in range(NCORES):
        bs = slice(c * BL, (c + 1) * BL)
        n_hat = (ALPHA * SIGMA) * noise[:, bs, :] + bias
        m = _pack_core(x[:, bs, :], n_hat.astype(np.float32), h0[bs], NSB)
        m["w_l"] = w_l
        in_maps.append(m)

    res = run_bass_kernel_spmd(nc, in_maps, list(range(NCORES)))

    out = np.empty((S, B, H), dtype=np.float32)
    for c in range(NCORES):
        out[:, c * BL : (c + 1) * BL, :] = _unpack_out(res.results[c]["out_l"], NSB)
    return out
